# revision 8
# baseline (speedup 1.0000x reference)
"""Trainium2 Bass kernel for windowed multi-head attention (2.5D swin-style).

Problem (hardcoded from spec nn_Attention25d_86775519248925):
  x:          (4, 16, 16, 8, 7, 7, 1, 128) f32  -> B=8192 windows, n=49 tokens, d=128
  w_qkv:      (128, 384) f32
  w_out:      (128, 128) f32
  bias_table: (169, 4) f32
  out:        same shape as x

Per window: qkv = x@w_qkv; per-head (h=4, dh=32) attention with relative
position bias; out = (softmax(q k^T/sqrt(dh) + bias) v) @ w_out.

Sharding: pure data parallel over the fused window-batch axis across 8 cores.

V2 layout strategy (per core, W windows; bf16 operands, fp32 PSUM):
  - host pre-pads x to 64-token window slots and casts to bf16; one xbar
    transpose-DMA per group loads x^T [d, 512 slots] directly (no PE
    transposes, no token-major staging). bf16 DMA out; host upcasts.
  - groups of 4 window-pairs (8 windows, 512 token slots).
  - q^T, k^T via shared-weight matmuls (w stationary, N=512 moving);
    v token-major (x^T pair slice stationary).
  - sim^T via 64-out tiles: per (pair, window, head) one 32x64 lhsT
    (k slice incl zero pad cols) x 32x49 rhs -> [64, 49],
    tile_position=(32h, 64w), bank-per-head PSUM (32 MMs/group).
  - softmax: attn_u = exp(sim) [ACT]; attn_b = attn_u * EB [POOL], where
    EB = exp(bias) with zeros in pad rows (also masks pads); Zb via a
    ones-block matmul (sums each 64-row block, broadcast over rows);
    1/Zb = exp(-ln Zb) [ACT x2, single shared act table set]; attn_n =
    attn_b * rzb [DVE, all-SBUF bf16].
  - attn@v: per (pair, window, head) one 64-contraction MM (64x32 lhsT =
    v slice, 64x49 rhs = attn_n) -> ps_y[32h:, w-bank, p, :49]; pad rows
    of attn_n are exactly zero so no partial-sum adds. 32 MMs/group.
  - final: lhsT = y^T pair slice stationary, rhs = w_out -> token-major
    out, copied to bf16 and DMA'd out.
  - activation-table thrash fix: the table chooser is restricted to
    natural_log_exp_and_others (contains exp, ln, copy) so the kernel
    performs exactly one ACT_TABLE_LOAD instead of four per group.

Hardware constraints (probed previously; CoreSim does not model them):
  - concurrent tile-position matmuls from different row-groups writing the
    same column-group must land in different PSUM banks (else
    NRT_EXEC_UNIT_UNRECOVERABLE); sim uses bank-per-head, attn@v
    bank-per-window.
  - PSUM accumulation chains spanning row-groups hang the device; all
    matmuls here are single-shot (start=True, stop=True).
"""

import os
import sys
import threading

import numpy as np

for _p in ("/opt/trn_rl_repo", "/root/.axon_site/_ro/trn_rl_repo"):
    if os.path.isdir(_p) and _p not in sys.path:
        sys.path.insert(0, _p)

# ---------------------------------------------------------------- constants
WS = 7
N_TOK = 49            # tokens per window
D = 128
H = 4
DH = 32
SCALE = DH ** -0.5
B_FULL = 4 * 16 * 16 * 8   # 8192 windows
N_CORES = 8


def _rel_pos_bias(bias_table: np.ndarray) -> np.ndarray:
    """bias[h, i, j] from the 169x4 table (numpy copy of reference logic)."""
    pos = np.arange(WS)
    gi, gj = np.meshgrid(pos, pos, indexing="ij")
    grid = np.stack([gi.reshape(-1), gj.reshape(-1)], axis=-1)
    rel = grid[:, None, :] - grid[None, :, :] + (WS - 1)
    idx = rel[..., 0] * (2 * WS - 1) + rel[..., 1]            # (49, 49)
    b = bias_table[idx]                                       # (49, 49, 4)
    return np.transpose(b, (2, 0, 1)).astype(np.float32)      # (h, i, j)


def _host_constants(w_qkv, w_out, bias_table):
    import ml_dtypes
    bf = ml_dtypes.bfloat16
    wq = np.ascontiguousarray((w_qkv[:, :D] * SCALE).astype(bf))
    wk = np.ascontiguousarray(w_qkv[:, D:2 * D].astype(bf))
    wv = np.ascontiguousarray(w_qkv[:, 2 * D:].astype(bf))
    wo = np.ascontiguousarray(w_out.astype(bf))

    bias = _rel_pos_bias(np.asarray(bias_table, dtype=np.float32))  # (h,i,j)
    # EB[64*wp + j, h, p, i] = exp(bias[h, i, j]) for j < 49, else 0.
    # Zero pad rows both apply the mask and make pad attn exactly 0.
    eb = np.zeros((128, H, 4, N_TOK), dtype=np.float32)
    expb = np.exp(bias)                                       # (h, i, j)
    for wp in range(2):
        for h in range(H):
            for p in range(4):
                eb[64 * wp: 64 * wp + N_TOK, h, p, :] = expb[h].T
    # ones-block matrix: obig[j, m] = 1 iff j and m are in the same 64-block
    ob = np.zeros((128, 128), dtype=bf)
    ob[:64, :64] = 1.0
    ob[64:, 64:] = 1.0
    return dict(wq=wq, wk=wk, wv=wv, wo=wo, eb=eb.astype(bf), obig=ob)


class _one_act_table:
    """Restrict the act-table chooser to natural_log_exp_and_others so
    exp/ln/copy share one set (one ACT_TABLE_LOAD total, no thrash).
    Set ids stay aligned with act_info.json (only contents are masked)."""

    KEEP = "natural_log_exp_and_others"

    def __enter__(self):
        import concourse.bacc as bacc_mod
        self._mod = bacc_mod
        self._orig = bacc_mod.get_activation_tables
        keep = self.KEEP

        def patched(arch):
            t = self._orig(arch)
            if keep not in t:          # fallback: leave untouched
                return t
            return {name: (s if name == keep else set())
                    for name, s in t.items()}

        bacc_mod.get_activation_tables = patched
        return self

    def __exit__(self, *exc):
        self._mod.get_activation_tables = self._orig
        return False


def _build_bass(n_windows: int):
    """Build the Bass/Tile program for one core processing n_windows windows."""
    import concourse.bacc as bacc
    import concourse.bass as bass
    import concourse.mybir as mybir
    import concourse.tile as tile

    f32 = mybir.dt.float32
    bf = mybir.dt.bfloat16
    NT = n_windows * N_TOK            # real tokens this core
    NS = n_windows * 64               # padded token slots
    n_pairs = n_windows // 2
    n_groups = n_pairs // 4
    assert n_windows % 8 == 0

    nc = bacc.Bacc("TRN2", target_bir_lowering=False, debug=False,
                   enable_asserts=False)

    x_in = nc.dram_tensor("x64", [NS, D], bf, kind="ExternalInput")
    out_t = nc.dram_tensor("out", [NT, D], bf, kind="ExternalOutput")
    wq_d = nc.dram_tensor("wq", [D, D], bf, kind="ExternalInput")
    wk_d = nc.dram_tensor("wk", [D, D], bf, kind="ExternalInput")
    wv_d = nc.dram_tensor("wv", [D, D], bf, kind="ExternalInput")
    wo_d = nc.dram_tensor("wo", [D, D], bf, kind="ExternalInput")
    eb_d = nc.dram_tensor("eb", [128, H, 4, N_TOK], bf, kind="ExternalInput")
    ob_d = nc.dram_tensor("obig", [128, 128], bf, kind="ExternalInput")

    Exp = mybir.ActivationFunctionType.Exp
    Ln = mybir.ActivationFunctionType.Ln

    with tile.TileContext(nc) as tc:
        with (
            tc.tile_pool(name="singles", bufs=1) as singles,
            tc.tile_pool(name="xt", bufs=4) as pool_xt,
            tc.tile_pool(name="qk", bufs=3) as pool_qk,
            tc.tile_pool(name="vsb", bufs=3) as pool_v,
            tc.tile_pool(name="attn", bufs=4) as pool_attn,
            tc.tile_pool(name="rz", bufs=4) as pool_rz,
            tc.tile_pool(name="ysb", bufs=3) as pool_y,
            tc.tile_pool(name="outb", bufs=3) as pool_out,
            tc.tile_pool(name="psA", bufs=1, space="PSUM") as pool_A,
            tc.tile_pool(name="psB", bufs=2, space="PSUM") as pool_B,
            tc.tile_pool(name="psY", bufs=1, space="PSUM") as pool_Y,
        ):
            wq_sb = singles.tile([D, D], bf, tag="wq")
            wk_sb = singles.tile([D, D], bf, tag="wk")
            wv_sb = singles.tile([D, D], bf, tag="wv")
            wo_sb = singles.tile([D, D], bf, tag="wo")
            eb_sb = singles.tile([128, H, 4, N_TOK], bf, tag="eb")
            ob_sb = singles.tile([128, 128], bf, tag="ob")
            for sb, dr in ((wq_sb, wq_d), (wk_sb, wk_d), (wv_sb, wv_d),
                           (wo_sb, wo_d), (eb_sb, eb_d), (ob_sb, ob_d)):
                nc.sync.dma_start(out=sb[:], in_=dr[:])

            for g in range(n_groups):
                tok0 = g * 392

                # ---- load x^T via xbar transpose DMA -----------------
                xT = pool_xt.tile([128, 4, D], bf, tag="xt")
                in_ap = bass.AP(tensor=x_in, offset=g * 512 * D,
                                ap=[[D, 512], [1, D]])
                nc.sync.dma_start(out=xT[:], in_=in_ap, transpose=True)

                # ---- q/k/v projections -------------------------------
                ps_q = pool_B.tile([128, 4, D], f32, tag="B")
                nc.tensor.matmul(ps_q[:], wq_sb[:], xT[:])
                qT = pool_qk.tile([128, 4, D], bf, tag="qT")
                nc.vector.tensor_copy(qT[:], ps_q[:])

                ps_k = pool_B.tile([128, 4, D], f32, tag="B")
                nc.tensor.matmul(ps_k[:], wk_sb[:], xT[:])
                kT = pool_qk.tile([128, 4, D], bf, tag="kT")
                nc.scalar.copy(kT[:], ps_k[:])

                ps_v = pool_B.tile([128, 4, D], f32, tag="B")
                for p in range(4):
                    nc.tensor.matmul(ps_v[:, p, :], xT[:, p, :], wv_sb[:])
                v_sb = pool_v.tile([128, 4, D], bf, tag="v")
                nc.vector.tensor_copy(v_sb[:], ps_v[:])

                # ---- attention: whole group in one softmax chain -----
                # sim^T: [j(2x64), h-bank, p, i]; one 64-row tile per
                # (p, w, h): lhsT = k slice (incl pad cols), rhs = q.
                ps_sim = pool_A.tile([128, H, 4, 128], f32, tag="A")
                for p in range(4):
                    for w_ in range(2):
                        for h in range(H):
                            lhsT = kT[32 * h: 32 * h + 32, p,
                                      64 * w_: 64 * w_ + 64]
                            rhs = qT[32 * h: 32 * h + 32, p,
                                     64 * w_: 64 * w_ + N_TOK]
                            o = ps_sim[64 * w_: 64 * w_ + 64, h, p, :N_TOK]
                            nc.tensor.matmul(
                                o, lhsT, rhs,
                                tile_position=(32 * h, 64 * w_),
                                start=True, stop=True)
                simv = ps_sim[:, :, :, :N_TOK]          # [128, h, p, 49]
                attn_u = pool_attn.tile([128, H, 4, N_TOK], bf, tag="attnU")
                nc.scalar.activation(attn_u[:], simv, Exp)
                # mask pads + apply exp(bias) in one multiply (POOL)
                attn_b = pool_attn.tile([128, H, 4, N_TOK], bf, tag="attnB")
                nc.gpsimd.tensor_mul(attn_b[:], attn_u[:], eb_sb[:])
                # Zb: per-64-block column sums broadcast to all rows
                # (two matmuls: moving free dim caps at 512 < 784)
                ps_zb = pool_Y.tile([128, 2, 512], f32, tag="Y")
                for s in range(2):
                    nc.tensor.matmul(ps_zb[:, s, :2 * H * N_TOK], ob_sb[:],
                                     attn_b[:, :, 2 * s: 2 * s + 2, :])
                lnzb = pool_rz.tile([128, 2, H, 2, N_TOK], bf, tag="lnzb")
                with nc.allow_low_precision(
                        reason="ln Z in bf16; Z in [20, 90], ok"):
                    nc.scalar.activation(
                        lnzb[:],
                        ps_zb[:, :, :2 * H * N_TOK].rearrange(
                            "a s (h q i) -> a s h q i", h=H, q=2),
                        Ln)
                rzb = pool_rz.tile([128, H, 4, N_TOK], bf, tag="rzb")
                with nc.allow_low_precision(
                        reason="1/Z via exp(-ln Z); bf16 ok"):
                    nc.scalar.activation(
                        rzb[:].rearrange("a h (s q) i -> a s h q i", s=2),
                        lnzb[:], Exp, scale=-1.0)
                attn_n = pool_attn.tile([128, H, 4, N_TOK], bf, tag="attnN")
                nc.vector.tensor_mul(attn_n[:], attn_b[:], rzb[:])

                # ---- attn @ v: one 64-contraction MM per (p, w, h) ----
                ps_y = pool_Y.tile([128, 2, 4, 128], f32, tag="Y")
                for p in range(4):
                    for w_ in range(2):
                        for h in range(H):
                            lhsT = v_sb[64 * w_: 64 * w_ + 64, p,
                                        32 * h: 32 * h + 32]
                            rhs = attn_n[64 * w_: 64 * w_ + 64, h, p, :]
                            o = ps_y[32 * h: 32 * h + 32, w_, p, :N_TOK]
                            nc.tensor.matmul(
                                o, lhsT, rhs,
                                tile_position=(64 * w_, 32 * h),
                                start=True, stop=True)
                y_sb = pool_y.tile([128, 4, 2, N_TOK], bf, tag="y")
                nc.vector.tensor_copy(
                    y_sb[:].rearrange("a p w i -> a w p i"),
                    ps_y[:, :, :, :N_TOK])

                # ---- output projection -------------------------------
                ps_f = pool_B.tile([2 * N_TOK, 4, D], f32, tag="B")
                for p in range(4):
                    nc.tensor.matmul(ps_f[:, p, :], y_sb[:, p, :, :],
                                     wo_sb[:])
                outb = pool_out.tile([2 * N_TOK, 4, D], bf, tag="outb")
                nc.scalar.copy(outb[:], ps_f[:])

                for p_ in range(2):
                    od_ap = bass.AP(
                        tensor=out_t, offset=(tok0 + p_ * N_TOK) * D,
                        ap=[[D, N_TOK], [2 * N_TOK * D, 4], [1, D]])
                    nc.sync.dma_start(
                        out=od_ap, in_=outb[N_TOK * p_: N_TOK * (p_ + 1)])

    with _one_act_table():
        nc.compile()
    return nc


# ------------------------------------------------------------- run helpers
_CACHE = {}
_LOCK = threading.Lock()
LAST_RESULT = None


def _get_nc(n_windows: int):
    with _LOCK:
        if n_windows not in _CACHE:
            _CACHE[n_windows] = _build_bass(n_windows)
        return _CACHE[n_windows]


def kernel(x, w_qkv, w_out, bias_table):
    import ml_dtypes
    from concourse.bass_utils import run_bass_kernel_spmd

    global LAST_RESULT
    bfd = ml_dtypes.bfloat16
    x = np.asarray(x, dtype=np.float32)
    b, X, Y, Z, w1, w2, w3, d = x.shape
    B = b * X * Y * Z
    assert B == B_FULL and w1 * w2 * w3 == N_TOK and d == D
    w_core = B // N_CORES

    consts = _host_constants(np.asarray(w_qkv, np.float32),
                             np.asarray(w_out, np.float32),
                             np.asarray(bias_table, np.float32))
    nc = _get_nc(w_core)

    # pad windows to 64-token slots, bf16
    x64 = np.zeros((B, 64, D), dtype=bfd)
    x64[:, :N_TOK, :] = x.reshape(B, N_TOK, D)
    x64 = x64.reshape(B * 64, D)
    ns = w_core * 64
    in_maps = []
    for c in range(N_CORES):
        m = {"x64": x64[c * ns: (c + 1) * ns],
             "wq": consts["wq"], "wk": consts["wk"], "wv": consts["wv"],
             "wo": consts["wo"], "eb": consts["eb"],
             "obig": consts["obig"]}
        in_maps.append(m)

    res = run_bass_kernel_spmd(nc, in_maps, core_ids=list(range(N_CORES)))
    LAST_RESULT = res
    out = np.concatenate([np.asarray(r["out"]) for r in res.results], axis=0)
    return out.astype(np.float32).reshape(x.shape)


# revision 11
# speedup vs baseline: 1.7077x; 1.7077x over previous
"""Trainium2 Bass kernel for windowed multi-head attention (2.5D swin-style).

Problem (hardcoded from spec nn_Attention25d_86775519248925):
  x:          (4, 16, 16, 8, 7, 7, 1, 128) f32  -> B=8192 windows, n=49 tokens, d=128
  w_qkv:      (128, 384) f32
  w_out:      (128, 128) f32
  bias_table: (169, 4) f32
  out:        same shape as x

Per window: qkv = x@w_qkv; per-head (h=4, dh=32) attention with relative
position bias; out = (softmax(q k^T/sqrt(dh) + bias) v) @ w_out.

Sharding: pure data parallel over the fused window-batch axis across 8 cores.

V2 layout strategy (per core, W windows; bf16 operands, fp32 PSUM):
  - host pre-pads x to 64-token window slots and casts to bf16; one xbar
    transpose-DMA per group loads x^T [d, 512 slots] directly (no PE
    transposes, no token-major staging). bf16 DMA out; host upcasts.
  - groups of 4 window-pairs (8 windows, 512 token slots).
  - q^T, k^T via shared-weight matmuls (w stationary, N=512 moving);
    v token-major (x^T pair slice stationary).
  - sim^T via 64-out tiles: per (pair, window, head) one 32x64 lhsT
    (k slice incl zero pad cols) x 32x49 rhs -> [64, 49],
    tile_position=(32h, 64w), bank-per-head PSUM (32 MMs/group).
  - softmax: attn_u = exp(sim) [ACT]; attn_b = attn_u * EB [POOL], where
    EB = exp(bias) with zeros in pad rows (also masks pads); Zb via a
    ones-block matmul (sums each 64-row block, broadcast over rows);
    1/Zb = exp(-ln Zb) [ACT x2, single shared act table set]; attn_n =
    attn_b * rzb [DVE, all-SBUF bf16].
  - attn@v: per (pair, window, head) one 64-contraction MM (64x32 lhsT =
    v slice, 64x49 rhs = attn_n) -> ps_y[32h:, w-bank, p, :49]; pad rows
    of attn_n are exactly zero so no partial-sum adds. 32 MMs/group.
  - final: lhsT = y^T pair slice stationary, rhs = w_out -> token-major
    out, copied to bf16 and DMA'd out.
  - activation-table thrash fix: the table chooser is restricted to
    natural_log_exp_and_others (contains exp, ln, copy) so the kernel
    performs exactly one ACT_TABLE_LOAD instead of four per group.

Hardware constraints (probed previously; CoreSim does not model them):
  - concurrent tile-position matmuls from different row-groups writing the
    same column-group must land in different PSUM banks (else
    NRT_EXEC_UNIT_UNRECOVERABLE); sim uses bank-per-head, attn@v
    bank-per-window.
  - PSUM accumulation chains spanning row-groups hang the device; all
    matmuls here are single-shot (start=True, stop=True).
"""

import os
import sys
import threading

import numpy as np

for _p in ("/opt/trn_rl_repo", "/root/.axon_site/_ro/trn_rl_repo"):
    if os.path.isdir(_p) and _p not in sys.path:
        sys.path.insert(0, _p)

# ---------------------------------------------------------------- constants
WS = 7
N_TOK = 49            # tokens per window
D = 128
H = 4
DH = 32
SCALE = DH ** -0.5
B_FULL = 4 * 16 * 16 * 8   # 8192 windows
N_CORES = 8


def _rel_pos_bias(bias_table: np.ndarray) -> np.ndarray:
    """bias[h, i, j] from the 169x4 table (numpy copy of reference logic)."""
    pos = np.arange(WS)
    gi, gj = np.meshgrid(pos, pos, indexing="ij")
    grid = np.stack([gi.reshape(-1), gj.reshape(-1)], axis=-1)
    rel = grid[:, None, :] - grid[None, :, :] + (WS - 1)
    idx = rel[..., 0] * (2 * WS - 1) + rel[..., 1]            # (49, 49)
    b = bias_table[idx]                                       # (49, 49, 4)
    return np.transpose(b, (2, 0, 1)).astype(np.float32)      # (h, i, j)


def _host_constants(w_qkv, w_out, bias_table):
    import ml_dtypes
    bf = ml_dtypes.bfloat16
    wq = np.ascontiguousarray((w_qkv[:, :D] * SCALE).astype(bf))
    wk = np.ascontiguousarray(w_qkv[:, D:2 * D].astype(bf))
    wv = np.ascontiguousarray(w_qkv[:, 2 * D:].astype(bf))
    wo = np.ascontiguousarray(w_out.astype(bf))

    bias = _rel_pos_bias(np.asarray(bias_table, dtype=np.float32))  # (h,i,j)
    # EB[64*wp + j, h, pp, i] = exp(bias[h, i, j]) for j < 49, else 0.
    # Zero pad rows both apply the mask and make pad attn exactly 0.
    eb = np.zeros((128, H, 2, N_TOK), dtype=np.float32)
    expb = np.exp(bias)                                       # (h, i, j)
    for wp in range(2):
        for h in range(H):
            eb[64 * wp: 64 * wp + N_TOK, h, 0, :] = expb[h].T
            eb[64 * wp: 64 * wp + N_TOK, h, 1, :] = expb[h].T
    # ones-block matrix: obig[j, m] = 1 iff j and m are in the same 64-block
    ob = np.zeros((128, 128), dtype=bf)
    ob[:64, :64] = 1.0
    ob[64:, 64:] = 1.0
    return dict(wq=wq, wk=wk, wv=wv, wo=wo, eb=eb.astype(bf), obig=ob)


class _one_act_table:
    """Restrict the act-table chooser to natural_log_exp_and_others so
    exp/ln/copy share one set (one ACT_TABLE_LOAD total, no thrash).
    Set ids stay aligned with act_info.json (only contents are masked)."""

    KEEP = "natural_log_exp_and_others"

    def __enter__(self):
        import concourse.bacc as bacc_mod
        self._mod = bacc_mod
        self._orig = bacc_mod.get_activation_tables
        keep = self.KEEP

        def patched(arch):
            t = self._orig(arch)
            if keep not in t:          # fallback: leave untouched
                return t
            return {name: (s if name == keep else set())
                    for name, s in t.items()}

        bacc_mod.get_activation_tables = patched
        return self

    def __exit__(self, *exc):
        self._mod.get_activation_tables = self._orig
        return False


def _build_bass(n_windows: int):
    """Build the Bass/Tile program for one core processing n_windows windows."""
    import concourse.bacc as bacc
    import concourse.bass as bass
    import concourse.mybir as mybir
    import concourse.tile as tile

    f32 = mybir.dt.float32
    bf = mybir.dt.bfloat16
    NT = n_windows * N_TOK            # real tokens this core
    NS = n_windows * 64               # padded token slots
    n_pairs = n_windows // 2
    n_groups = n_pairs // 4
    assert n_windows % 8 == 0

    nc = bacc.Bacc("TRN2", target_bir_lowering=False, debug=False,
                   enable_asserts=False)

    x_in = nc.dram_tensor("x64", [NS, D], bf, kind="ExternalInput")
    out_t = nc.dram_tensor("out", [NT, D], bf, kind="ExternalOutput")
    wq_d = nc.dram_tensor("wq", [D, D], bf, kind="ExternalInput")
    wk_d = nc.dram_tensor("wk", [D, D], bf, kind="ExternalInput")
    wv_d = nc.dram_tensor("wv", [D, D], bf, kind="ExternalInput")
    wo_d = nc.dram_tensor("wo", [D, D], bf, kind="ExternalInput")
    eb_d = nc.dram_tensor("eb", [128, H, 2, N_TOK], bf, kind="ExternalInput")
    ob_d = nc.dram_tensor("obig", [128, 128], bf, kind="ExternalInput")

    Exp = mybir.ActivationFunctionType.Exp
    Ln = mybir.ActivationFunctionType.Ln

    with tile.TileContext(nc) as tc:
        with (
            tc.tile_pool(name="singles", bufs=1) as singles,
            tc.tile_pool(name="xt", bufs=4) as pool_xt,
            tc.tile_pool(name="qk", bufs=3) as pool_qk,
            tc.tile_pool(name="vsb", bufs=3) as pool_v,
            tc.tile_pool(name="attn", bufs=6) as pool_attn,
            tc.tile_pool(name="rz", bufs=6) as pool_rz,
            tc.tile_pool(name="ysb", bufs=3) as pool_y,
            tc.tile_pool(name="outb", bufs=3) as pool_out,
            tc.tile_pool(name="psA", bufs=1, space="PSUM") as pool_A,
            tc.tile_pool(name="psB", bufs=2, space="PSUM") as pool_B,
            tc.tile_pool(name="psY", bufs=1, space="PSUM") as pool_Y,
        ):
            wq_sb = singles.tile([D, D], bf, tag="wq")
            wk_sb = singles.tile([D, D], bf, tag="wk")
            wv_sb = singles.tile([D, D], bf, tag="wv")
            wo_sb = singles.tile([D, D], bf, tag="wo")
            eb_sb = singles.tile([128, H, 2, N_TOK], bf, tag="eb")
            ob_sb = singles.tile([128, 128], bf, tag="ob")
            for sb, dr in ((wq_sb, wq_d), (wk_sb, wk_d), (wv_sb, wv_d),
                           (wo_sb, wo_d), (eb_sb, eb_d), (ob_sb, ob_d)):
                nc.sync.dma_start(out=sb[:], in_=dr[:])

            for g in range(n_groups):
                tok0 = g * 392

                # ---- load x^T via xbar transpose DMA -----------------
                xT = pool_xt.tile([128, 4, D], bf, tag="xt")
                in_ap = bass.AP(tensor=x_in, offset=g * 512 * D,
                                ap=[[D, 512], [1, D]])
                nc.sync.dma_start(out=xT[:], in_=in_ap, transpose=True)

                # ---- q/k/v projections -------------------------------
                ps_q = pool_B.tile([128, 4, D], f32, tag="B")
                nc.tensor.matmul(ps_q[:], wq_sb[:], xT[:])
                qT = pool_qk.tile([128, 4, D], bf, tag="qT")
                nc.vector.tensor_copy(qT[:], ps_q[:])

                ps_k = pool_B.tile([128, 4, D], f32, tag="B")
                nc.tensor.matmul(ps_k[:], wk_sb[:], xT[:])
                kT = pool_qk.tile([128, 4, D], bf, tag="kT")
                nc.vector.tensor_copy(kT[:], ps_k[:])

                ps_v = pool_B.tile([128, 4, D], f32, tag="B")
                for p in range(4):
                    nc.tensor.matmul(ps_v[:, p, :], xT[:, p, :], wv_sb[:])
                v_sb = pool_v.tile([128, 4, D], bf, tag="v")
                nc.vector.tensor_copy(v_sb[:], ps_v[:])

                # ---- attention ---------------------------------------
                attn_tiles = []
                for sp in range(2):
                    # sim^T: [j(2x64), h-bank, pp, i]; one 64-row tile per
                    # (pp, w, h): lhsT = k slice (incl pad cols), rhs = q.
                    ps_sim = pool_A.tile([128, H, 2, 256], f32, tag="A")
                    for pp in range(2):
                        p = sp * 2 + pp
                        for w_ in range(2):
                            for h in range(H):
                                lhsT = kT[32 * h: 32 * h + 32, p,
                                          64 * w_: 64 * w_ + 64]
                                rhs = qT[32 * h: 32 * h + 32, p,
                                         64 * w_: 64 * w_ + N_TOK]
                                o = ps_sim[64 * w_: 64 * w_ + 64, h, pp,
                                           :N_TOK]
                                nc.tensor.matmul(
                                    o, lhsT, rhs,
                                    tile_position=(32 * h, 64 * w_),
                                    start=True, stop=True)
                    simv = ps_sim[:, :, :, :N_TOK]      # [128, h, pp, 49]
                    attn_u = pool_attn.tile([128, H, 2, N_TOK], bf,
                                            tag="attnU")
                    nc.scalar.activation(attn_u[:], simv, Exp)
                    # mask pads + apply exp(bias) in one multiply (POOL)
                    attn_b = pool_attn.tile([128, H, 2, N_TOK], bf,
                                            tag="attnB")
                    nc.gpsimd.tensor_mul(attn_b[:], attn_u[:], eb_sb[:])
                    # Zb: per-64-block column sums broadcast to all rows
                    ps_zbt = pool_Y.tile([128, 2, 4, 128], f32,
                                         name=f"zb{g}_{sp}", tag="Y")
                    ps_zb = ps_zbt[:, 0, :, :].rearrange("a b c -> a (b c)")
                    nc.tensor.matmul(ps_zb[:, :2 * H * N_TOK], ob_sb[:],
                                     attn_b[:])
                    lnzb = pool_rz.tile([128, 2 * H * N_TOK], bf,
                                        tag="lnzb")
                    with nc.allow_low_precision(
                            reason="ln Z in bf16; Z in [20, 90], ok"):
                        nc.scalar.activation(lnzb[:],
                                             ps_zb[:, :2 * H * N_TOK], Ln)
                    rzb = pool_rz.tile([128, H, 2, N_TOK], bf, tag="rzb")
                    with nc.allow_low_precision(
                            reason="1/Z via exp(-ln Z); bf16 ok"):
                        nc.scalar.activation(
                            rzb[:],
                            lnzb[:].rearrange("a (h q i) -> a h q i",
                                              h=H, q=2),
                            Exp, scale=-1.0)
                    attn_n = pool_attn.tile([128, H, 2, N_TOK], bf,
                                            tag="attnN")
                    if sp == 0:
                        nc.vector.tensor_mul(attn_n[:], attn_b[:], rzb[:])
                    else:
                        nc.gpsimd.tensor_mul(attn_n[:], attn_b[:], rzb[:])
                    attn_tiles.append(attn_n)

                # ---- attn @ v: one 64-contraction MM per (p, w, h) ----
                ps_y = pool_Y.tile([128, 2, 4, 128], f32, tag="Y")
                for p in range(4):
                    attn_n = attn_tiles[p // 2]
                    pp = p % 2
                    for w_ in range(2):
                        for h in range(H):
                            lhsT = v_sb[64 * w_: 64 * w_ + 64, p,
                                        32 * h: 32 * h + 32]
                            rhs = attn_n[64 * w_: 64 * w_ + 64, h, pp, :]
                            o = ps_y[32 * h: 32 * h + 32, w_, p, :N_TOK]
                            nc.tensor.matmul(
                                o, lhsT, rhs,
                                tile_position=(64 * w_, 32 * h),
                                start=True, stop=True)
                y_sb = pool_y.tile([128, 4, 2, N_TOK], bf, tag="y")
                nc.vector.tensor_copy(
                    y_sb[:].rearrange("a p w i -> a w p i"),
                    ps_y[:, :, :, :N_TOK])

                # ---- output projection -------------------------------
                ps_ft = pool_Y.tile([2 * N_TOK, 2, 4, 128], f32,
                                    name=f"f{g}", tag="Y")
                ps_f = ps_ft[:, 0, :, :]
                for p in range(4):
                    nc.tensor.matmul(ps_f[:, p, :], y_sb[:, p, :, :],
                                     wo_sb[:])
                outb = pool_out.tile([2 * N_TOK, 4, D], bf, tag="outb")
                nc.vector.tensor_copy(outb[:], ps_f[:])

                for p_ in range(2):
                    od_ap = bass.AP(
                        tensor=out_t, offset=(tok0 + p_ * N_TOK) * D,
                        ap=[[D, N_TOK], [2 * N_TOK * D, 4], [1, D]])
                    nc.sync.dma_start(
                        out=od_ap, in_=outb[N_TOK * p_: N_TOK * (p_ + 1)])

    with _one_act_table():
        nc.compile()
    return nc


# ------------------------------------------------------------- run helpers
_CACHE = {}
_LOCK = threading.Lock()
LAST_RESULT = None


def _get_nc(n_windows: int):
    with _LOCK:
        if n_windows not in _CACHE:
            _CACHE[n_windows] = _build_bass(n_windows)
        return _CACHE[n_windows]


def kernel(x, w_qkv, w_out, bias_table):
    import ml_dtypes
    from concourse.bass_utils import run_bass_kernel_spmd

    global LAST_RESULT
    bfd = ml_dtypes.bfloat16
    x = np.asarray(x, dtype=np.float32)
    b, X, Y, Z, w1, w2, w3, d = x.shape
    B = b * X * Y * Z
    assert B == B_FULL and w1 * w2 * w3 == N_TOK and d == D
    w_core = B // N_CORES

    consts = _host_constants(np.asarray(w_qkv, np.float32),
                             np.asarray(w_out, np.float32),
                             np.asarray(bias_table, np.float32))
    nc = _get_nc(w_core)

    # pad windows to 64-token slots, bf16
    x64 = np.zeros((B, 64, D), dtype=bfd)
    x64[:, :N_TOK, :] = x.reshape(B, N_TOK, D)
    x64 = x64.reshape(B * 64, D)
    ns = w_core * 64
    in_maps = []
    for c in range(N_CORES):
        m = {"x64": x64[c * ns: (c + 1) * ns],
             "wq": consts["wq"], "wk": consts["wk"], "wv": consts["wv"],
             "wo": consts["wo"], "eb": consts["eb"],
             "obig": consts["obig"]}
        in_maps.append(m)

    res = run_bass_kernel_spmd(nc, in_maps, core_ids=list(range(N_CORES)))
    LAST_RESULT = res
    out = np.concatenate([np.asarray(r["out"]) for r in res.results], axis=0)
    return out.astype(np.float32).reshape(x.shape)
